# revision 1
# baseline (speedup 1.0000x reference)
"""Trainium2 Bass kernel for nn_DecoderLSTM_noAttention (greedy decode LSTM).

Strategy (8 NeuronCores, SPMD, all-fp32 numerics):
- Vocab-sharded FC: each core holds a 4000-column slice of W_fc.T and
  computes its logits slice each step in 8 PSUM-bank-sized chunks; each chunk
  is DMA'd straight from PSUM to the output and fed to DVE max/max_index
  (overlapped under the remaining FC matmuls).
- The input-side gate contributions are precomputed on the host as
  gtab = embedding @ W_ih.T + (b_ih + b_hh)  [32000, 2048], so a step's
  gates need only one indirect row-gather plus the on-chip h @ W_hh.T.
  The W_hh matmul for step t+1 runs during step t's argmax exchange,
  keeping the PE warm.
- Greedy argmax: per-chunk top-8, on-chip combine (first-occurrence
  tie-breaks), then an AllGather of (best value, global id) per batch row;
  every core picks the global winner deterministically.
- Batch-sharded phase 0 for the encoder mean + AllGather of summaries.
- Output: each core writes logits [31, 64, 4000]; the host assembles the
  full [64, 32, 32000] (t=0 stays zero).
"""
import numpy as np

import concourse.bass as bass
import concourse.bacc as bacc
import concourse.tile as tile
from concourse import mybir
from concourse.bass_utils import run_bass_kernel_spmd
from concourse.masks import make_identity

F32 = mybir.dt.float32
F16 = mybir.dt.float16
I32 = mybir.dt.int32
U32 = mybir.dt.uint32
AF = mybir.ActivationFunctionType
OP = mybir.AluOpType

B = 64          # batch
H = 512         # hidden = embed
V = 32000       # vocab
T = 32          # max_len
NPIX = 196
NCORES = 8
BL = B // NCORES      # local batch (phase 0)
VL = V // NCORES      # local vocab slice
NSTEPS = T - 1
GD = 2048             # gate dim

# FC chunking: bank-sized
CHUNKS = [(q * 512, 512) for q in range(7)] + [(3584, 416)]
NQ = len(CHUNKS)

_CACHE = {}


def _build_nc(nsteps=NSTEPS, out_slots=NSTEPS, no_cc=False, no_max=False):
    nc = bacc.Bacc("TRN2", target_bir_lowering=False, debug=False, num_devices=NCORES)

    # ---- DRAM parameters ----
    gtab_d = nc.dram_tensor("gtab", [V, GD], F32, kind="ExternalInput")
    whh_d = nc.dram_tensor("whhT", [4, 128, GD], F32, kind="ExternalInput")
    wfchi_d = nc.dram_tensor("wfcThi", [4, 128, VL], F16, kind="ExternalInput")
    wfclo_d = nc.dram_tensor("wfcTlo", [4, 128, VL], F16, kind="ExternalInput")
    winit_d = nc.dram_tensor("winitT", [4, 128, 1024], F32, kind="ExternalInput")
    bfc_d = nc.dram_tensor("bfc", [1, VL], F32, kind="ExternalInput")
    binit_d = nc.dram_tensor("binit", [1, 1024], F32, kind="ExternalInput")
    enc_d = nc.dram_tensor("enc", [13, 128, H], F32, kind="ExternalInput")
    blk_d = nc.dram_tensor("blkdiag", [128, 13 * 8], F32, kind="ExternalInput")
    tok0_d = nc.dram_tensor("tok0", [B, 1], I32, kind="ExternalInput")
    vbase_d = nc.dram_tensor("vbase", [B, 1], F32, kind="ExternalInput")

    out_d = nc.dram_tensor("logits", [out_slots, B, VL], F32, kind="ExternalOutput")

    with tile.TileContext(nc) as tc:
        import contextlib
        with contextlib.ExitStack() as ctx:
            const = ctx.enter_context(tc.tile_pool(name="const", bufs=1))
            work = ctx.enter_context(tc.tile_pool(name="work", bufs=1))
            hc = ctx.enter_context(tc.tile_pool(name="hc", bufs=2))
            small = ctx.enter_context(tc.tile_pool(name="small", bufs=2))
            lgp = ctx.enter_context(tc.tile_pool(name="lgp", bufs=1))
            ptr = ctx.enter_context(tc.tile_pool(name="ptr", bufs=1, space="PSUM"))
            pg = ctx.enter_context(tc.tile_pool(name="pg", bufs=1, space="PSUM"))
            pfc = ctx.enter_context(tc.tile_pool(name="pfc", bufs=2, space="PSUM"))
            dram = ctx.enter_context(tc.tile_pool(name="dram", bufs=2, space="DRAM"))
            dramsh = ctx.enter_context(
                tc.tile_pool(name="dramsh", bufs=2, space="DRAM"))

            # ---- constants / weights into SBUF ----
            ident = const.tile([B, B], F32)
            make_identity(nc, ident[:])
            ones1 = const.tile([1, B], F32)
            nc.vector.memset(ones1[:], 1.0)
            vb64 = const.tile([B, 1], F32)
            nc.sync.dma_start(vb64[:], vbase_d[:])
            qbase = const.tile([B, NQ * 8], F32)
            for q, (off, _w) in enumerate(CHUNKS):
                nc.vector.memset(qbase[:, q * 8:(q + 1) * 8], float(off))

            whh = []
            for k in range(4):
                w = const.tile([128, GD], F32, tag=f"whh{k}")
                nc.sync.dma_start(w[:], whh_d[k])
                whh.append(w)
            wfchi, wfclo = [], []
            for k in range(4):
                w = const.tile([128, VL], F16, tag=f"wfchi{k}")
                nc.sync.dma_start(w[:], wfchi_d[k])
                wfchi.append(w)
                w = const.tile([128, VL], F16, tag=f"wfclo{k}")
                nc.sync.dma_start(w[:], wfclo_d[k])
                wfclo.append(w)
            bfc = const.tile([1, VL], F32)
            nc.sync.dma_start(bfc[:], bfc_d[:])
            binit = const.tile([1, 1024], F32)
            nc.sync.dma_start(binit[:], binit_d[:])
            blk = work.tile([128, 13 * 8], F32, tag="gx")
            nc.sync.dma_start(blk[:], blk_d[:])

            def transpose_to(src, dst_tile):
                """src: SBUF [B, 512] fp32 -> dst SBUF [128, 4*B] feature-major."""
                for k in range(4):
                    pt = ptr.tile([128, B], F32, tag="ptr")
                    nc.tensor.transpose(
                        out=pt[:], in_=src[:, k * 128:(k + 1) * 128],
                        identity=ident[:])
                    nc.scalar.copy(dst_tile[:, k * B:(k + 1) * B], pt[:])

            # ================= phase 0 =================
            psum0 = pg.tile([BL, H], F32, tag="pg")
            for k in range(13):
                et = work.tile([128, H], F32, tag="gsum")
                nc.sync.dma_start(et[:], enc_d[k])
                nc.tensor.matmul(
                    psum0[:], lhsT=blk[:, k * 8:(k + 1) * 8], rhs=et[:],
                    start=(k == 0), stop=(k == 12))
            sums = work.tile([BL, H], F32, tag="tng")
            nc.scalar.copy(sums[:], psum0[:])

            sumfull = work.tile([B, H], F32, tag="sgo")
            if no_cc:
                for ci in range(NCORES):
                    nc.vector.tensor_copy(sumfull[ci * BL:(ci + 1) * BL, :], sums[:])
            else:
                cc0_in = dram.tile([BL, H], F32, tag="cc0i")
                cc0_out = dramsh.tile([NCORES, BL, H], F32, addr_space="Shared",
                                      tag="cc0o")
                nc.sync.dma_start(cc0_in[:], sums[:])
                nc.gpsimd.collective_compute(
                    "AllGather", OP.bypass,
                    replica_groups=[list(range(NCORES))],
                    ins=[cc0_in[:]], outs=[cc0_out[:]])
                nc.sync.dma_start(sumfull[:], cc0_out[:].rearrange("c b h -> (c b) h"))

            sumT = work.tile([128, 4 * B], F32, tag="hT")
            transpose_to(sumfull, sumT)

            for n in range(2):
                ph = pfc.tile([B, 512], F32, tag="pfc")
                for k in range(4):
                    wi = work.tile([128, 1024], F32, tag="sigif")
                    nc.sync.dma_start(wi[:], winit_d[k])
                    nc.tensor.matmul(
                        ph[:], lhsT=sumT[:, k * B:(k + 1) * B],
                        rhs=wi[:, n * 512:(n + 1) * 512],
                        start=(k == 0), stop=False)
                nc.tensor.matmul(
                    ph[:], lhsT=ones1[:],
                    rhs=binit[:, n * 512:(n + 1) * 512],
                    start=False, stop=True)
                dst = hc.tile([B, H], F32, tag=("h" if n == 0 else "c"))
                nc.scalar.copy(dst[:], ph[:])
                if n == 0:
                    h_cur = dst
                else:
                    c_cur = dst
            hT = work.tile([128, 4 * B], F32, tag="hT")
            transpose_to(h_cur, hT)
            hhiT = work.tile([128, 4 * B], F16, tag="hhiT")
            nc.vector.tensor_copy(hhiT[:], hT[:])
            hres = work.tile([128, 4 * B], F32, tag="hres")
            nc.vector.tensor_sub(hres[:], hT[:], hhiT[:])
            hloT = work.tile([128, 4 * B], F16, tag="hloT")
            nc.vector.tensor_copy(hloT[:], hres[:])

            tok = small.tile([B, 1], I32, tag="tok")
            nc.sync.dma_start(tok[:], tok0_d[:])

            # ================= decode steps =================
            for t in range(nsteps):
                # --- gather input-gate contributions (gtab rows) ---
                gx = work.tile([B, GD], F32, tag="gx")
                nc.gpsimd.indirect_dma_start(
                    out=gx[:], out_offset=None, in_=gtab_d[:],
                    in_offset=bass.IndirectOffsetOnAxis(ap=tok[:, :1], axis=0))

                # --- h @ W_hh.T into PSUM (ready early: overlaps exchange) ---
                pgt = pg.tile([B, GD], F32, tag="pg")
                for k in range(4):
                    for n in range(4):
                        nc.tensor.matmul(
                            pgt[:, n * 512:(n + 1) * 512],
                            lhsT=hT[:, k * B:(k + 1) * B],
                            rhs=whh[k][:, n * 512:(n + 1) * 512],
                            start=(k == 0), stop=(k == 3))

                # --- gates = psum + gx; pointwise LSTM ---
                gsum = work.tile([B, GD], F32, tag="gsum")
                nc.vector.tensor_add(gsum[:], pgt[:], gx[:])
                sig_if = work.tile([B, 1024], F32, tag="sigif")
                nc.scalar.activation(sig_if[:], gsum[:, 0:1024], AF.Sigmoid)
                tng = work.tile([B, 512], F32, tag="tng")
                nc.scalar.activation(tng[:], gsum[:, 1024:1536], AF.Tanh)
                sgo = work.tile([B, 512], F32, tag="sgo")
                nc.scalar.activation(sgo[:], gsum[:, 1536:2048], AF.Sigmoid)

                t1 = work.tile([B, 512], F32, tag="t1")
                nc.vector.tensor_mul(t1[:], sig_if[:, 0:512], tng[:])
                t2 = work.tile([B, 512], F32, tag="t2")
                nc.vector.tensor_mul(t2[:], sig_if[:, 512:1024], c_cur[:])
                c_new = hc.tile([B, H], F32, tag="c")
                nc.vector.tensor_add(c_new[:], t2[:], t1[:])
                tc2 = work.tile([B, 512], F32, tag="tc2")
                nc.scalar.activation(tc2[:], c_new[:], AF.Tanh)
                h_new = hc.tile([B, H], F32, tag="h")
                nc.vector.tensor_mul(h_new[:], sgo[:], tc2[:])
                c_cur = c_new

                hT = work.tile([128, 4 * B], F32, tag="hT")
                transpose_to(h_new, hT)
                hhiT = work.tile([128, 4 * B], F16, tag="hhiT")
                nc.vector.tensor_copy(hhiT[:], hT[:])
                hres = work.tile([128, 4 * B], F32, tag="hres")
                nc.vector.tensor_sub(hres[:], hT[:], hhiT[:])
                hloT = work.tile([128, 4 * B], F16, tag="hloT")
                nc.vector.tensor_copy(hloT[:], hres[:])

                # --- FC in bank chunks; ACT copy to SBUF; per-chunk top-8 ---
                cands = small.tile([B, NQ * 8], F32, tag="cands")
                cidx = small.tile([B, NQ * 8], U32, tag="cidx")
                logits = lgp.tile([B, VL], F32, tag="logits")
                for q, (off, w) in enumerate(CHUNKS):
                    pf = pfc.tile([B, 512], F32, tag="pfc")
                    for k in range(4):
                        nc.tensor.matmul(
                            pf[:, :w], lhsT=hhiT[:, k * B:(k + 1) * B],
                            rhs=wfchi[k][:, off:off + w],
                            start=(k == 0), stop=False)
                    for k in range(4):
                        nc.tensor.matmul(
                            pf[:, :w], lhsT=hloT[:, k * B:(k + 1) * B],
                            rhs=wfchi[k][:, off:off + w],
                            start=False, stop=False)
                        nc.tensor.matmul(
                            pf[:, :w], lhsT=hhiT[:, k * B:(k + 1) * B],
                            rhs=wfclo[k][:, off:off + w],
                            start=False, stop=False)
                    nc.tensor.matmul(
                        pf[:, :w], lhsT=ones1[:], rhs=bfc[:, off:off + w],
                        start=False, stop=True)
                    lg = logits[:, off:off + w]
                    nc.scalar.copy(lg, pf[:, :w])
                    nc.sync.dma_start(out_d[t][:, off:off + w], lg)
                    if not no_max:
                        nc.vector.max(out=cands[:, q * 8:(q + 1) * 8], in_=lg)
                        nc.vector.max_index(
                            out=cidx[:, q * 8:(q + 1) * 8],
                            in_max=cands[:, q * 8:(q + 1) * 8], in_values=lg)

                # --- combine chunk winners (exact, first-occurrence ties) ---
                pack = small.tile([B, 2], F32, tag="pack")
                if no_max:
                    nc.vector.tensor_scalar(
                        out=pack[:, 0:1], in0=hT[:B, 0:1], scalar1=0.0,
                        scalar2=None, op0=OP.mult)
                    nc.vector.tensor_copy(pack[:, 1:2], pack[:, 0:1])
                else:
                    wv = small.tile([B, 8], F32, tag="wv")
                    nc.vector.max(out=wv[:], in_=cands[:])
                    msk = small.tile([B, NQ * 8], F32, tag="msk")
                    nc.vector.tensor_scalar(
                        out=msk[:], in0=cands[:], scalar1=wv[:, 0:1], scalar2=None,
                        op0=OP.is_equal)
                    idxf = small.tile([B, NQ * 8], F32, tag="idxf")
                    nc.vector.tensor_copy(idxf[:], cidx[:])
                    gidx = small.tile([B, NQ * 8], F32, tag="gidx")
                    nc.vector.tensor_add(gidx[:], idxf[:], qbase[:])
                    gneg = small.tile([B, NQ * 8], F32, tag="gneg")
                    nc.vector.tensor_scalar(
                        out=gneg[:], in0=gidx[:], scalar1=-1.0, scalar2=48000.0,
                        op0=OP.mult, op1=OP.add)
                    gsel = small.tile([B, NQ * 8], F32, tag="gsel")
                    nc.vector.tensor_mul(gsel[:], msk[:], gneg[:])
                    w2 = small.tile([B, 8], F32, tag="w2")
                    nc.vector.max(out=w2[:], in_=gsel[:])
                    # local best value / global id
                    nc.vector.tensor_copy(pack[:, 0:1], wv[:, 0:1])
                    lid = small.tile([B, 1], F32, tag="lid")
                    nc.vector.tensor_scalar(
                        out=lid[:], in0=w2[:, 0:1], scalar1=-1.0, scalar2=48000.0,
                        op0=OP.mult, op1=OP.add)
                    nc.vector.tensor_add(pack[:, 1:2], lid[:], vb64[:])

                # --- exchange + global winner ---
                if no_cc:
                    arr = small.tile([B, 16], F32, tag="arr")
                    for cci in range(8):
                        nc.vector.tensor_copy(
                            arr[:, :].rearrange("b (c j) -> b c j", j=2)[:, cci],
                            pack[:])
                else:
                    cc_in = dram.tile([B, 2], F32, tag="cci")
                    cc_out = dramsh.tile([NCORES, B, 2], F32, addr_space="Shared",
                                         tag="cco")
                    nc.sync.dma_start(cc_in[:], pack[:])
                    nc.gpsimd.collective_compute(
                        "AllGather", OP.bypass,
                        replica_groups=[list(range(NCORES))],
                        ins=[cc_in[:]], outs=[cc_out[:]])
                    arr = small.tile([B, 16], F32, tag="arr")
                    nc.sync.dma_start(
                        arr[:, :].rearrange("b (c j) -> b c j", j=2),
                        cc_out[:].rearrange("c b j -> b c j"))

                vals = arr[:, :].rearrange("b (c j) -> b c j", j=2)[:, :, 0]
                gids = arr[:, :].rearrange("b (c j) -> b c j", j=2)[:, :, 1]
                wmax = small.tile([B, 8], F32, tag="wmax")
                nc.vector.max(out=wmax[:], in_=vals)
                msk2 = small.tile([B, 8], F32, tag="msk2")
                nc.vector.tensor_scalar(
                    out=msk2[:], in0=vals, scalar1=wmax[:, 0:1], scalar2=None,
                    op0=OP.is_equal)
                gneg2 = small.tile([B, 8], F32, tag="gneg2")
                nc.vector.tensor_scalar(
                    out=gneg2[:], in0=gids, scalar1=-1.0, scalar2=40000.0,
                    op0=OP.mult, op1=OP.add)
                gsel2 = small.tile([B, 8], F32, tag="gsel2")
                nc.vector.tensor_mul(gsel2[:], msk2[:], gneg2[:])
                w22 = small.tile([B, 8], F32, tag="w22")
                nc.vector.max(out=w22[:], in_=gsel2[:])
                tokf = small.tile([B, 1], F32, tag="tokf")
                nc.vector.tensor_scalar(
                    out=tokf[:], in0=w22[:, 0:1], scalar1=-1.0, scalar2=40000.0,
                    op0=OP.mult, op1=OP.add)
                tok = small.tile([B, 1], I32, tag="tok")
                nc.vector.tensor_copy(tok[:], tokf[:])

    nc.compile()
    return nc


def _prep_inputs(inputs):
    enc = np.ascontiguousarray(np.asarray(inputs["encoder_outputs"], np.float32))
    captions = np.asarray(inputs["captions"])
    emb = np.asarray(inputs["embedding"], np.float32)
    W_ih = np.asarray(inputs["W_ih"], np.float32)
    b_ih = np.asarray(inputs["b_ih"], np.float32)
    W_hh = np.asarray(inputs["W_hh"], np.float32)
    b_hh = np.asarray(inputs["b_hh"], np.float32)
    W_fc = np.asarray(inputs["W_fc"], np.float32)
    b_fc = np.asarray(inputs["b_fc"], np.float32)
    W_init_h = np.asarray(inputs["W_init_h"], np.float32)
    b_init_h = np.asarray(inputs["b_init_h"], np.float32)
    W_init_c = np.asarray(inputs["W_init_c"], np.float32)
    b_init_c = np.asarray(inputs["b_init_c"], np.float32)

    gtab = (emb @ W_ih.T + (b_ih + b_hh)).astype(np.float32)
    whhT = np.ascontiguousarray(W_hh.T.reshape(4, 128, GD))
    winitT = np.ascontiguousarray(
        (np.concatenate([W_init_h, W_init_c], axis=0) / np.float32(NPIX))
        .T.reshape(4, 128, 1024))
    binit = np.concatenate([b_init_h, b_init_c]).reshape(1, 1024)
    tok0 = np.ascontiguousarray(captions[:, 0].astype(np.int32).reshape(B, 1))

    blk = np.zeros((128, 13 * 8), np.float32)
    for k in range(13):
        for i in range(128):
            r = k * 128 + i
            if r < BL * NPIX:
                blk[i, k * 8 + r // NPIX] = 1.0

    in_maps = []
    for c in range(NCORES):
        enc_c = enc[c * BL:(c + 1) * BL].reshape(BL * NPIX, H)
        enc_pad = np.zeros((13 * 128, H), np.float32)
        enc_pad[:BL * NPIX] = enc_c
        wfc_slice = W_fc[c * VL:(c + 1) * VL]
        wfcT = wfc_slice.T.astype(np.float32)
        wfcT_hi = wfcT.astype(np.float16)
        wfcT_lo = (wfcT - wfcT_hi.astype(np.float32)).astype(np.float16)
        in_maps.append({
            "gtab": gtab,
            "whhT": whhT,
            "wfcThi": np.ascontiguousarray(wfcT_hi.reshape(4, 128, VL)),
            "wfcTlo": np.ascontiguousarray(wfcT_lo.reshape(4, 128, VL)),
            "winitT": winitT,
            "bfc": np.ascontiguousarray(b_fc[c * VL:(c + 1) * VL].reshape(1, VL)),
            "binit": binit,
            "enc": enc_pad.reshape(13, 128, H),
            "blkdiag": blk,
            "tok0": tok0,
            "vbase": np.full((B, 1), c * VL, np.float32),
        })
    return in_maps


def kernel(**inputs) -> np.ndarray:
    if "nc" not in _CACHE:
        _CACHE["nc"] = _build_nc()
    nc = _CACHE["nc"]
    in_maps = _prep_inputs(inputs)
    res = run_bass_kernel_spmd(nc, in_maps, list(range(NCORES)))
    out = np.zeros((B, T, V), np.float32)
    for c in range(NCORES):
        lg = res.results[c]["logits"][:NSTEPS]     # [31, 64, VL]
        out[:, 1:, c * VL:(c + 1) * VL] = lg.transpose(1, 0, 2)
    return out



# revision 13
# speedup vs baseline: 1.5394x; 1.5394x over previous
"""Trainium2 Bass kernel for nn_DecoderLSTM_noAttention (greedy decode LSTM).

Strategy (8 NeuronCores, SPMD, zero collectives):
- Data-parallel over batch: core c owns batch rows [8c, 8c+8) end-to-end.
  The sequential 31-step greedy decode runs fully per-core; no cross-core
  communication at all (collectives on this axon per-core topology are
  host-emulated and cost ~270ms each).
- LSTM gates on-chip in fp32: x rows gathered from the embedding table by
  indirect DMA, then x@W_ih.T + h@W_hh.T + (b_ih+b_hh) accumulated in one
  PSUM group.
- FC (h @ W_fc.T + b_fc) is memory-bound: W_fc.T is streamed from DRAM in
  fp16 every step (32.8 MB/step in 16 double-buffered 2MB slabs).  Logits
  are written out in fp16 (abs err ~3e-4 << tolerance).
- Greedy argmax must match the fp32 reference ordering exactly, so the
  fp16 scan only nominates candidates: per-4000-column segment top-8
  (DVE max/max_index), then 4 knockout rounds pick the scan top-4.  The 4
  candidate rows of the original fp32 W_fc (+bias) are indirect-gathered
  and their exact fp32 dots with the full-precision h decide the token.
- Output: each core writes fp16 logits [31, 8, 32000]; the host assembles
  the full fp32 [64, 32, 32000] (t=0 stays zero).
"""
import numpy as np

import concourse.bass as bass
import concourse.bacc as bacc
import concourse.tile as tile
from concourse import mybir
from concourse.bass_utils import run_bass_kernel_spmd
from concourse.masks import make_identity

F32 = mybir.dt.float32
F16 = mybir.dt.float16
I32 = mybir.dt.int32
U16 = mybir.dt.uint16
AF = mybir.ActivationFunctionType
OP = mybir.AluOpType

B = 64          # global batch
H = 512         # hidden = embed
V = 32000       # vocab
T = 32          # max_len
NPIX = 196
NCORES = 8
BL = B // NCORES      # local batch rows per core
NSTEPS = T - 1
GD = 2048             # gate dim

NSLAB = 32            # W_fc.T stream slabs per step
SLABW = V // NSLAB    # 1000 columns per slab
NSEG = 8              # scan-max segments
SEGW = V // NSEG      # 4000 columns per segment
NCAND = 4             # exact-recheck candidates
WROWW = 520           # padded fp32 W_fc row (512 weights + bias + pad)

# chunk offsets within one slab (PSUM bank is 512 fp32 wide)
SLAB_CHUNKS = [(0, 512), (512, 488)]

_CACHE = {}


def _build_nc(nsteps=NSTEPS, out_slots=NSTEPS):
    nc = bacc.Bacc("TRN2", target_bir_lowering=False, debug=False,
                   num_devices=NCORES)

    # ---- DRAM parameters ----
    emb_d = nc.dram_tensor("emb", [V, H], F32, kind="ExternalInput")
    wrow_d = nc.dram_tensor("wrow", [V, WROWW], F32, kind="ExternalInput")
    whi_d = nc.dram_tensor("whiT", [NSLAB, 128, 4, SLABW], F16,
                           kind="ExternalInput")
    wih_d = nc.dram_tensor("wihT", [4, 128, GD], F32, kind="ExternalInput")
    whh_d = nc.dram_tensor("whhT", [4, 128, GD], F32, kind="ExternalInput")
    winit_d = nc.dram_tensor("winitT", [4, 128, 1024], F32, kind="ExternalInput")
    bgate_d = nc.dram_tensor("bgate", [1, GD], F32, kind="ExternalInput")
    binit_d = nc.dram_tensor("binit", [1, 1024], F32, kind="ExternalInput")
    enc_d = nc.dram_tensor("enc", [13, 128, H], F32, kind="ExternalInput")
    blk_d = nc.dram_tensor("blkdiag", [128, 13 * 8], F32, kind="ExternalInput")
    tok0_d = nc.dram_tensor("tok0", [BL, 1], I32, kind="ExternalInput")

    out_d = nc.dram_tensor("logits", [out_slots, BL, V], F16,
                           kind="ExternalOutput")

    with tile.TileContext(nc) as tc:
        import contextlib
        with contextlib.ExitStack() as ctx:
            const = ctx.enter_context(tc.tile_pool(name="const", bufs=1))
            work = ctx.enter_context(tc.tile_pool(name="work", bufs=1))
            hc = ctx.enter_context(tc.tile_pool(name="hc", bufs=2))
            small = ctx.enter_context(tc.tile_pool(name="small", bufs=2))
            lgp = ctx.enter_context(tc.tile_pool(name="lgp", bufs=1))
            stream = ctx.enter_context(tc.tile_pool(name="stream", bufs=2))
            ptr = ctx.enter_context(tc.tile_pool(name="ptr", bufs=1, space="PSUM"))
            pg = ctx.enter_context(tc.tile_pool(name="pg", bufs=1, space="PSUM"))
            pfc = ctx.enter_context(tc.tile_pool(name="pfc", bufs=2, space="PSUM"))

            # ---- constants / weights into SBUF ----
            ident = const.tile([BL, BL], F32)
            make_identity(nc, ident[:])
            ones1 = const.tile([1, BL], F32)
            nc.vector.memset(ones1[:], 1.0)
            segbase = const.tile([BL, NSEG * 8], F32)
            for j in range(NSEG):
                nc.vector.memset(segbase[:, j * 8:(j + 1) * 8], float(j * SEGW))

            wih = []
            whh = []
            for k in range(4):
                w = const.tile([128, GD], F32, tag=f"wih{k}")
                nc.sync.dma_start(w[:], wih_d[k])
                wih.append(w)
                w = const.tile([128, GD], F32, tag=f"whh{k}")
                nc.sync.dma_start(w[:], whh_d[k])
                whh.append(w)
            bgate = const.tile([1, GD], F32)
            nc.sync.dma_start(bgate[:], bgate_d[:])
            binit = const.tile([1, 1024], F32)
            nc.sync.dma_start(binit[:], binit_d[:])
            blk = work.tile([128, 13 * 8], F32, tag="blk")
            nc.sync.dma_start(blk[:], blk_d[:])

            def transpose_to(src, dst_tile):
                """src: SBUF [BL, 512] fp32 -> dst SBUF [128, 4*BL]."""
                for k in range(4):
                    pt = ptr.tile([128, BL], F32, tag="ptr")
                    nc.tensor.transpose(
                        out=pt[:], in_=src[:, k * 128:(k + 1) * 128],
                        identity=ident[:])
                    nc.scalar.copy(dst_tile[:, k * BL:(k + 1) * BL], pt[:])

            # ================= phase 0: h0/c0 from encoder mean =================
            psum0 = pfc.tile([BL, H], F32, tag="pfc")
            for k in range(13):
                et = work.tile([128, H], F32, tag="enc")
                nc.sync.dma_start(et[:], enc_d[k])
                nc.tensor.matmul(
                    psum0[:], lhsT=blk[:, k * 8:(k + 1) * 8], rhs=et[:],
                    start=(k == 0), stop=(k == 12))
            sums = work.tile([BL, H], F32, tag="sums")
            nc.scalar.copy(sums[:], psum0[:])

            sumT = work.tile([128, 4 * BL], F32, tag="hT")
            transpose_to(sums, sumT)

            for n in range(2):
                ph = pfc.tile([BL, 512], F32, tag="pfc")
                for k in range(4):
                    wi = work.tile([128, 1024], F32, tag="winit")
                    nc.sync.dma_start(wi[:], winit_d[k])
                    nc.tensor.matmul(
                        ph[:], lhsT=sumT[:, k * BL:(k + 1) * BL],
                        rhs=wi[:, n * 512:(n + 1) * 512],
                        start=(k == 0), stop=False)
                nc.tensor.matmul(
                    ph[:], lhsT=ones1[:],
                    rhs=binit[:, n * 512:(n + 1) * 512],
                    start=False, stop=True)
                dst = hc.tile([BL, H], F32, tag=("h" if n == 0 else "c"))
                nc.scalar.copy(dst[:], ph[:])
                if n == 0:
                    h_cur = dst
                else:
                    c_cur = dst
            hT = work.tile([128, 4 * BL], F32, tag="hT")
            transpose_to(h_cur, hT)

            tok = small.tile([BL, 1], I32, tag="tok")
            nc.sync.dma_start(tok[:], tok0_d[:])

            # ================= decode steps =================
            for t in range(nsteps):
                # --- x = embedding[tok]  (indirect row gather) ---
                x = work.tile([BL, H], F32, tag="x")
                nc.gpsimd.indirect_dma_start(
                    out=x[:], out_offset=None, in_=emb_d[:],
                    in_offset=bass.IndirectOffsetOnAxis(ap=tok[:, :1], axis=0))
                xT = work.tile([128, 4 * BL], F32, tag="xT")
                transpose_to(x, xT)

                # --- gates = x@W_ih.T + h@W_hh.T + (b_ih+b_hh) in one PSUM ---
                pgt = pg.tile([BL, GD], F32, tag="pg")
                for n in range(4):
                    ns = slice(n * 512, (n + 1) * 512)
                    for k in range(4):
                        nc.tensor.matmul(
                            pgt[:, ns], lhsT=hT[:, k * BL:(k + 1) * BL],
                            rhs=whh[k][:, ns], start=(k == 0), stop=False)
                    for k in range(4):
                        nc.tensor.matmul(
                            pgt[:, ns], lhsT=xT[:, k * BL:(k + 1) * BL],
                            rhs=wih[k][:, ns], start=False, stop=False)
                    nc.tensor.matmul(
                        pgt[:, ns], lhsT=ones1[:], rhs=bgate[:, ns],
                        start=False, stop=True)

                # --- pointwise LSTM ---
                sig_if = work.tile([BL, 1024], F32, tag="sigif")
                nc.scalar.activation(sig_if[:], pgt[:, 0:1024], AF.Sigmoid)
                tng = work.tile([BL, 512], F32, tag="tng")
                nc.scalar.activation(tng[:], pgt[:, 1024:1536], AF.Tanh)
                sgo = work.tile([BL, 512], F32, tag="sgo")
                nc.scalar.activation(sgo[:], pgt[:, 1536:2048], AF.Sigmoid)

                t1 = work.tile([BL, 512], F32, tag="t1")
                nc.vector.tensor_mul(t1[:], sig_if[:, 0:512], tng[:])
                t2 = work.tile([BL, 512], F32, tag="t2")
                nc.vector.tensor_mul(t2[:], sig_if[:, 512:1024], c_cur[:])
                c_new = hc.tile([BL, H], F32, tag="c")
                nc.vector.tensor_add(c_new[:], t2[:], t1[:])
                tc2 = work.tile([BL, 512], F32, tag="tc2")
                nc.scalar.activation(tc2[:], c_new[:], AF.Tanh)
                h_new = hc.tile([BL, H], F32, tag="h")
                nc.vector.tensor_mul(h_new[:], sgo[:], tc2[:])
                c_cur = c_new

                hT = work.tile([128, 4 * BL], F32, tag="hT")
                transpose_to(h_new, hT)
                hhiT = work.tile([128, 4 * BL], F16, tag="hhiT")
                nc.vector.tensor_copy(hhiT[:], hT[:])

                # --- FC scan: stream W_fc.T fp16, logits to L, top-8/segment ---
                L = lgp.tile([BL, V], F16, tag="L")
                cands = small.tile([BL, NSEG * 8], F16, tag="cands")
                cidx = small.tile([BL, NSEG * 8], U16, tag="cidx")
                for s in range(NSLAB):
                    st = stream.tile([128, 4, SLABW], F16, tag="st")
                    nc.sync.dma_start(st[:], whi_d[s])
                    base = s * SLABW
                    for off, w in SLAB_CHUNKS:
                        pf = pfc.tile([BL, 512], F32, tag="pfc")
                        for k in range(4):
                            nc.tensor.matmul(
                                pf[:, :w], lhsT=hhiT[:, k * BL:(k + 1) * BL],
                                rhs=st[:, k, off:off + w],
                                start=(k == 0), stop=(k == 3))
                        nc.scalar.copy(L[:, base + off:base + off + w],
                                       pf[:, :w])
                    if (s + 1) % (NSLAB // NSEG) == 0:
                        j = (s + 1) // (NSLAB // NSEG) - 1
                        js = slice(j * 8, (j + 1) * 8)
                        nc.vector.max(out=cands[:, js],
                                      in_=L[:, j * SEGW:(j + 1) * SEGW])
                        nc.vector.max_index(
                            out=cidx[:, js], in_max=cands[:, js],
                            in_values=L[:, j * SEGW:(j + 1) * SEGW])
                nc.sync.dma_start(out_d[min(t, out_slots - 1)][:], L[:])

                # --- scan top-4 candidates (knockout on [BL, 64]) ---
                cv = small.tile([BL, NSEG * 8], F32, tag="cv")
                nc.vector.tensor_copy(cv[:], cands[:])
                gidxf = small.tile([BL, NSEG * 8], F32, tag="gidxf")
                nc.vector.tensor_copy(gidxf[:], cidx[:])
                nc.vector.tensor_add(gidxf[:], gidxf[:], segbase[:])
                gneg = small.tile([BL, NSEG * 8], F32, tag="gneg")
                nc.vector.tensor_scalar(
                    out=gneg[:], in0=gidxf[:], scalar1=-1.0, scalar2=48000.0,
                    op0=OP.mult, op1=OP.add)
                gid4 = small.tile([BL, NCAND], F32, tag="gid4")
                for r in range(NCAND):
                    m8 = small.tile([BL, 8], F32, tag="m8")
                    nc.vector.max(out=m8[:], in_=cv[:])
                    msk = small.tile([BL, NSEG * 8], F32, tag="msk")
                    nc.vector.tensor_scalar(
                        out=msk[:], in0=cv[:], scalar1=m8[:, 0:1], scalar2=None,
                        op0=OP.is_equal)
                    sel = small.tile([BL, NSEG * 8], F32, tag="sel")
                    nc.vector.tensor_mul(sel[:], msk[:], gneg[:])
                    w8 = small.tile([BL, 8], F32, tag="w8")
                    nc.vector.max(out=w8[:], in_=sel[:])
                    nc.vector.tensor_scalar(
                        out=gid4[:, r:r + 1], in0=w8[:, 0:1], scalar1=-1.0,
                        scalar2=48000.0, op0=OP.mult, op1=OP.add)
                    if r + 1 < NCAND:
                        oh = small.tile([BL, NSEG * 8], F32, tag="oh")
                        nc.vector.tensor_scalar(
                            out=oh[:], in0=gneg[:], scalar1=w8[:, 0:1],
                            scalar2=None, op0=OP.is_equal)
                        nc.vector.tensor_scalar(
                            out=oh[:], in0=oh[:], scalar1=-1e30, scalar2=None,
                            op0=OP.mult)
                        nc.vector.tensor_add(cv[:], cv[:], oh[:])

                # --- exact fp32 recheck of the 4 candidates ---
                cid = small.tile([BL, NCAND], I32, tag="cid")
                nc.vector.tensor_copy(cid[:], gid4[:])
                wcand = work.tile([BL, NCAND, WROWW], F32, tag="wcand")
                for j in range(NCAND):
                    nc.gpsimd.indirect_dma_start(
                        out=wcand[:, j], out_offset=None, in_=wrow_d[:],
                        in_offset=bass.IndirectOffsetOnAxis(
                            ap=cid[:, j:j + 1], axis=0))
                ex = small.tile([BL, 8], F32, tag="ex")
                nc.vector.memset(ex[:], -1e30)
                prod = work.tile([BL, 512], F32, tag="prod")
                for j in range(NCAND):
                    nc.vector.tensor_mul(prod[:], wcand[:, j, 0:512], h_new[:])
                    nc.vector.tensor_reduce(
                        out=ex[:, j:j + 1], in_=prod[:],
                        axis=mybir.AxisListType.X, op=OP.add)
                for j in range(NCAND):
                    nc.vector.tensor_add(ex[:, j:j + 1], ex[:, j:j + 1],
                                         wcand[:, j, 512:513])

                # --- winner (value desc, then smallest id) -> next token ---
                em8 = small.tile([BL, 8], F32, tag="em8")
                nc.vector.max(out=em8[:], in_=ex[:])
                emsk = small.tile([BL, NCAND], F32, tag="emsk")
                nc.vector.tensor_scalar(
                    out=emsk[:], in0=ex[:, 0:NCAND], scalar1=em8[:, 0:1],
                    scalar2=None, op0=OP.is_equal)
                egneg = small.tile([BL, NCAND], F32, tag="egneg")
                nc.vector.tensor_scalar(
                    out=egneg[:], in0=gid4[:], scalar1=-1.0, scalar2=48000.0,
                    op0=OP.mult, op1=OP.add)
                esel = small.tile([BL, 8], F32, tag="esel")
                nc.vector.memset(esel[:], 0.0)
                nc.vector.tensor_mul(esel[:, 0:NCAND], emsk[:], egneg[:])
                ew8 = small.tile([BL, 8], F32, tag="ew8")
                nc.vector.max(out=ew8[:], in_=esel[:])
                tokf = small.tile([BL, 1], F32, tag="tokf")
                nc.vector.tensor_scalar(
                    out=tokf[:], in0=ew8[:, 0:1], scalar1=-1.0, scalar2=48000.0,
                    op0=OP.mult, op1=OP.add)
                tok = small.tile([BL, 1], I32, tag="tok")
                nc.vector.tensor_copy(tok[:], tokf[:])

    nc.compile()
    return nc


def _prep_inputs(inputs):
    enc = np.ascontiguousarray(np.asarray(inputs["encoder_outputs"], np.float32))
    captions = np.asarray(inputs["captions"])
    emb = np.ascontiguousarray(np.asarray(inputs["embedding"], np.float32))
    W_ih = np.asarray(inputs["W_ih"], np.float32)
    b_ih = np.asarray(inputs["b_ih"], np.float32)
    W_hh = np.asarray(inputs["W_hh"], np.float32)
    b_hh = np.asarray(inputs["b_hh"], np.float32)
    W_fc = np.asarray(inputs["W_fc"], np.float32)
    b_fc = np.asarray(inputs["b_fc"], np.float32)
    W_init_h = np.asarray(inputs["W_init_h"], np.float32)
    b_init_h = np.asarray(inputs["b_init_h"], np.float32)
    W_init_c = np.asarray(inputs["W_init_c"], np.float32)
    b_init_c = np.asarray(inputs["b_init_c"], np.float32)

    wrow = np.zeros((V, WROWW), np.float32)
    wrow[:, 0:512] = W_fc
    wrow[:, 512] = b_fc
    wT16 = W_fc.T.astype(np.float16)                       # [512, V]
    whi = np.ascontiguousarray(
        wT16.reshape(4, 128, NSLAB, SLABW).transpose(2, 1, 0, 3))
    wihT = np.ascontiguousarray(W_ih.T.reshape(4, 128, GD))
    whhT = np.ascontiguousarray(W_hh.T.reshape(4, 128, GD))
    winitT = np.ascontiguousarray(
        (np.concatenate([W_init_h, W_init_c], axis=0) / np.float32(NPIX))
        .T.reshape(4, 128, 1024))
    bgate = (b_ih + b_hh).reshape(1, GD).astype(np.float32)
    binit = np.concatenate([b_init_h, b_init_c]).reshape(1, 1024)

    blkd = np.zeros((128, 13 * 8), np.float32)
    for k in range(13):
        for i in range(128):
            r = k * 128 + i
            if r < BL * NPIX:
                blkd[i, k * 8 + r // NPIX] = 1.0

    in_maps = []
    for c in range(NCORES):
        enc_c = enc[c * BL:(c + 1) * BL].reshape(BL * NPIX, H)
        enc_pad = np.zeros((13 * 128, H), np.float32)
        enc_pad[:BL * NPIX] = enc_c
        tok0 = np.ascontiguousarray(
            captions[c * BL:(c + 1) * BL, 0].astype(np.int32).reshape(BL, 1))
        in_maps.append({
            "emb": emb,
            "wrow": wrow,
            "whiT": whi,
            "wihT": wihT,
            "whhT": whhT,
            "winitT": winitT,
            "bgate": bgate,
            "binit": binit,
            "enc": enc_pad.reshape(13, 128, H),
            "blkdiag": blkd,
            "tok0": tok0,
        })
    return in_maps


def kernel(**inputs) -> np.ndarray:
    if "nc" not in _CACHE:
        _CACHE["nc"] = _build_nc()
    nc = _CACHE["nc"]
    in_maps = _prep_inputs(inputs)
    res = run_bass_kernel_spmd(nc, in_maps, list(range(NCORES)))
    out = np.zeros((B, T, V), np.float32)
    for c in range(NCORES):
        lg = res.results[c]["logits"][:NSTEPS]     # [31, BL, V] fp16
        out[c * BL:(c + 1) * BL, 1:, :] = lg.transpose(1, 0, 2)
    b_fc = np.asarray(inputs["b_fc"], np.float32)
    if b_fc.any():
        # scan logits are computed without the FC bias (it is ordering-exact
        # in the on-chip recheck); fold it into the full output here
        out[:, 1:, :] += b_fc
    return out
